# revision 12
# baseline (speedup 1.0000x reference)
"""Trainium2 Bass kernel for nn_GroupedConvFuseSide4.

out[b,k] = w[k,0]*side5[b,k] + w[k,1]*side4[b,k]
         + w[k,2]*side1[b,0] + w[k,3]*side2[b,0] + w[k,4]*side3[b,0] + bias[k]

Sharding: pure data parallel over batch (B=8) across 8 NeuronCores.

v6 scheme — fp16 wire, half-tile streaming, PE-heavy pipeline:
  Pixels of one image are split into CH=32 chunks of FD=8192.  Row
  r = 19*g + k gives ROWS=608 rows; 5 row-tiles (4x128 + 96) x 2
  column halves stream through SBUF as [R, 4096] 1 MB transfers with
  5-deep pools, so the free->load->use chain never starves compute
  (v5's 2 MB/3-buf tiles made loads compute-paced in the second half).
  The three singles + a ones row are RESIDENT: one [97, FD] SBUF tile
  (row 32*s+g, row 96 = ones for bias) loaded once.  Per 512-col region:
    - PE: diag(w1) @ x4  (start=True) + lsT @ [singles; ones] (stop=True).
      Both stationary operands are 128-col slices of ONE global
      [128, 1232] fp16 const blob (LS | GD | f32-scalars-bitcast), a
      single ~2.4 KB/partition DMA (v5 streamed 3 small-descriptor
      consts that clogged the HWDGE queues for ~10 us at startup).
    - DVE: one STT per [R, 2048] block: out = x5*w0 + psum (measured
      2.34 us/block; a single full pass is the DVE minimum since STT
      never gets the 2x packed mode).
  A 10-matmul warmup burst runs at kernel start so the PE HAM clock
  gate flips to 2.4 GHz before the real matmuls begin (cold PE runs
  matmuls at 634 ns vs 379 ns warm / ~218 ns issue rate).
  DMA queues balanced ~9.9 MB each: sync = blob + all x5 loads,
  scalar = all x4 loads + t4 64-part stores, gpsimd = singles +
  half-tile 1 MB stores + t4 32-part stores.  Loads are never queued
  behind stores on any queue.  All DMA partition counts are in
  {1, 32, 64, 128} (counts in 65..127 hit degenerate descriptor paths).
  Host converts to fp16 and repacks so every load is contiguous;
  output comes back fp16, upcast on host.
  Max rel err vs the f32 reference ~8e-4, well under the 2e-2 gate.
"""

import numpy as np

B, K, H, W = 8, 19, 512, 512
FD = 8192                  # pixels per chunk
CH = 32                    # chunks per image (H*W / FD)
ROWS = K * CH              # 608 packed rows per core
TILES = []                 # (row0, nrows): 4 x 128 + 1 x 96
_r = 0
while _r < ROWS:
    TILES.append((_r, min(128, ROWS - _r)))
    _r += 128
NT = len(TILES)
HW_ = FD // 2             # half-tile columns (4096)
N_CORES = 8
BLOB_W = 1232             # 608 LS + 608 GD + 10 scl-f32-raw + 6 pad

_cache = {}


def _build_program(w, b):
    import concourse.bacc as bacc
    import concourse.tile as tile
    import concourse.mybir as mybir
    from contextlib import ExitStack

    f16 = mybir.dt.float16
    f32 = mybir.dt.float32
    mult = mybir.AluOpType.mult
    add = mybir.AluOpType.add

    nc = bacc.Bacc(
        "TRN2", target_bir_lowering=False, debug=False,
        enable_asserts=False, num_devices=N_CORES,
    )

    x5_d = nc.dram_tensor("x5", [ROWS, FD], f16, kind="ExternalInput").ap()
    x4_d = nc.dram_tensor("x4", [ROWS, FD], f16, kind="ExternalInput").ap()
    s_d = nc.dram_tensor("s", [97, FD], f16, kind="ExternalInput").ap()
    out_d = nc.dram_tensor("out", [ROWS, FD], f16, kind="ExternalOutput").ap()

    # ---- one const blob; per-tile lhsT = 128-col slices of it ----
    pp = np.arange(ROWS)
    kk_g = pp % K
    gg_g = pp // K
    blob = np.zeros((128, BLOB_W), dtype=np.float16)
    # LS[32*s + g(p), p] = w[k(p), 2+s]; LS[96, p] = bias[k(p)]
    for s in range(3):
        blob[32 * s + gg_g, pp] = w[kk_g, 2 + s].astype(np.float16)
    blob[96, pp] = b[kk_g].astype(np.float16)
    # GD[r, p] = w1[k(p)] iff r == p % 128 (per-tile diagonal), at col 608+p
    blob[pp % 128, 608 + pp] = w[kk_g, 1].astype(np.float16)
    # w0 per-partition f32 scalars, raw-bitcast into fp16 cols 1216:1226
    scl = np.zeros((128, NT), dtype=np.float32)
    for t, (r0, R) in enumerate(TILES):
        rr = r0 + np.arange(R)
        scl[:R, t] = w[rr % K, 0]
    blob[:, 1216:1216 + 2 * NT] = scl.view(np.float16)
    blob_dram = nc.inline_tensor(blob, name="blobc").ap()

    with tile.TileContext(nc) as tc, ExitStack() as ctx:
        consts = ctx.enter_context(tc.tile_pool(name="consts", bufs=1))
        x5_pool = ctx.enter_context(tc.tile_pool(name="x5", bufs=5))
        x4_pool = ctx.enter_context(tc.tile_pool(name="x4", bufs=5))
        o_pool = ctx.enter_context(tc.tile_pool(name="o", bufs=3))
        o4_pool = ctx.enter_context(tc.tile_pool(name="o4", bufs=4))
        ps_pool = ctx.enter_context(tc.tile_pool(name="ps", bufs=2, space="PSUM"))

        # blob + singles ride the gpsimd queue so the x5/x4 streams lead the
        # HWDGE queues from the first instruction (the blob's small
        # descriptors otherwise stall the sync queue through the slow
        # early-DMA phase).
        blob_t = consts.tile([128, BLOB_W], f16, tag="blob")
        nc.gpsimd.dma_start(out=blob_t[:], in_=blob_dram)
        s_t = consts.tile([97, FD], f16, tag="s")
        nc.gpsimd.dma_start(out=s_t[0:64], in_=s_d[0:64])
        nc.gpsimd.dma_start(out=s_t[64:96], in_=s_d[64:96])
        nc.gpsimd.dma_start(out=s_t[96:97], in_=s_d[96:97])
        scl_t = blob_t[:, 1216:1216 + 2 * NT].bitcast(f32)

        # ---- PE warmup: dense matmuls so the HAM clock gate flips to
        # 2.4 GHz before the first real matmul; results are dead writes.
        wu = consts.tile([128, 512], f16, tag="wu")
        nc.vector.memset(wu[:], 0.0)
        ps_w = ps_pool.tile([128, 2048], f32, tag="ps", name="ps_warm")
        for i in range(10):
            nc.tensor.matmul(
                ps_w[:, 0:512], wu[:, 0:128], wu[:],
                start=True, stop=True, skip_group_check=True,
            )

        # process the fiddly 96-row tail tile mid-stream (its small split
        # DMAs ride the DMA-bound middle's slack) so the kernel ends on
        # full-width tiles, and the last tile's stores fan out across the
        # by-then idle HWDGE queues.
        order = [0, 1, 4, 2, 3] if NT == 5 else list(range(NT))
        for t in order:
            r0, R = TILES[t]
            last = t == order[-1]
            diagT = blob_t[0:R, 608 + r0:608 + r0 + R]
            lsT = blob_t[0:97, r0:r0 + R]
            w0 = scl_t[0:R, t:t + 1]
            for half in range(2):
                c0 = HW_ * half
                x5h = x5_pool.tile([R, HW_], f16, tag="x5", name=f"x5_{t}_{half}")
                x4h = x4_pool.tile([R, HW_], f16, tag="x4", name=f"x4_{t}_{half}")
                if R == 128:
                    nc.sync.dma_start(out=x5h[:], in_=x5_d[r0:r0 + R, c0:c0 + HW_])
                    nc.scalar.dma_start(out=x4h[:], in_=x4_d[r0:r0 + R, c0:c0 + HW_])
                else:  # 96-row tail: keep DMA partition counts in {64,32}
                    nc.sync.dma_start(out=x5h[0:64],
                                      in_=x5_d[r0:r0 + 64, c0:c0 + HW_])
                    nc.sync.dma_start(out=x5h[64:R],
                                      in_=x5_d[r0 + 64:r0 + R, c0:c0 + HW_])
                    nc.scalar.dma_start(out=x4h[0:64],
                                        in_=x4_d[r0:r0 + 64, c0:c0 + HW_])
                    nc.scalar.dma_start(out=x4h[64:R],
                                        in_=x4_d[r0 + 64:r0 + R, c0:c0 + HW_])
                oh = (o_pool.tile([R, HW_], f16, tag="o", name=f"o_{t}_{half}")
                      if R == 128 else None)

                for sub in range(2):
                    lc = 2048 * sub           # local col in half tile
                    gc = c0 + lc              # global col
                    ps = ps_pool.tile([R, 2048], f32, tag="ps",
                                      name=f"ps_{t}_{half}_{sub}")
                    # all 4 ls matmuls back-to-back (need only the resident
                    # singles, so they can run before x4 lands), then all 4
                    # diag matmuls: consecutive matmuls share the stationary
                    # operand so the PE pays 2 weight swaps per block instead
                    # of 8 (a swap costs ~170ns of array refill per matmul).
                    for h in range(4):
                        nc.tensor.matmul(
                            ps[:, 512 * h:512 * h + 512], lsT,
                            s_t[0:97, gc + 512 * h:gc + 512 * h + 512],
                            start=True, stop=False, skip_group_check=True,
                        )
                    for h in range(4):
                        nc.tensor.matmul(
                            ps[:, 512 * h:512 * h + 512], diagT,
                            x4h[:, lc + 512 * h:lc + 512 * h + 512],
                            start=False, stop=True, skip_group_check=True,
                        )
                    if oh is not None:
                        nc.vector.scalar_tensor_tensor(
                            oh[:, lc:lc + 2048], x5h[:, lc:lc + 2048], w0,
                            ps[:], mult, add)
                    else:
                        o4 = o4_pool.tile([R, 2048], f16, tag="o4",
                                          name=f"o4_{half}_{sub}")
                        nc.vector.scalar_tensor_tensor(
                            o4[:], x5h[:, lc:lc + 2048], w0, ps[:], mult, add)
                        # tail tile: store each block immediately, split
                        # across the (by now idle) other queues
                        nc.scalar.dma_start(
                            out=out_d[r0:r0 + 64, gc:gc + 2048], in_=o4[0:64])
                        nc.gpsimd.dma_start(
                            out=out_d[r0 + 64:r0 + R, gc:gc + 2048],
                            in_=o4[64:R])
                if oh is not None:
                    if last:
                        nc.sync.dma_start(
                            out=out_d[r0:r0 + 64, c0:c0 + HW_], in_=oh[0:64])
                        nc.scalar.dma_start(
                            out=out_d[r0 + 64:r0 + R, c0:c0 + HW_],
                            in_=oh[64:R])
                    else:
                        nc.gpsimd.dma_start(out=out_d[r0:r0 + R, c0:c0 + HW_],
                                            in_=oh[:])

    nc.compile()
    return nc


def _get_program(w, b):
    key = (w.tobytes(), b.tobytes())
    if key not in _cache:
        _cache[key] = _build_program(w, b)
    return _cache[key]


def _pack_kchw(a16):
    """[K, CH, FD] fp16 -> [ROWS, FD], row = 19*g + k."""
    return np.ascontiguousarray(a16.transpose(1, 0, 2)).reshape(ROWS, FD)


def run(inputs, trace=False, tmpdir=None):
    from concourse.bass_utils import run_bass_kernel_spmd

    w = np.asarray(inputs["weight"], dtype=np.float32)
    b = np.asarray(inputs["bias"], dtype=np.float32)
    nc = _get_program(w, b)

    s1h = np.asarray(inputs["side1"]).astype(np.float16).reshape(B, CH, FD)
    s2h = np.asarray(inputs["side2"]).astype(np.float16).reshape(B, CH, FD)
    s3h = np.asarray(inputs["side3"]).astype(np.float16).reshape(B, CH, FD)
    s4h = np.asarray(inputs["side4"]).astype(np.float16).reshape(B, K, CH, FD)
    s5h = np.asarray(inputs["side5"]).astype(np.float16).reshape(B, K, CH, FD)

    in_maps = []
    for c in range(N_CORES):
        sp = np.empty((97, FD), dtype=np.float16)
        sp[0:32] = s1h[c]
        sp[32:64] = s2h[c]
        sp[64:96] = s3h[c]
        sp[96] = np.float16(1.0)
        in_maps.append({
            "x5": _pack_kchw(s5h[c]),
            "x4": _pack_kchw(s4h[c]),
            "s": sp,
        })

    res = run_bass_kernel_spmd(nc, in_maps, list(range(N_CORES)),
                               trace=trace, tmpdir=tmpdir)
    outs = []
    for c in range(N_CORES):
        o = res.results[c]["out"].reshape(CH, K, FD).transpose(1, 0, 2)
        outs.append(o.reshape(1, K, H, W).astype(np.float32))
    return np.concatenate(outs, axis=0), res


def kernel(**inputs):
    out, _ = run(inputs, trace=False)
    return out


# revision 14
# speedup vs baseline: 1.1421x; 1.1421x over previous
"""Trainium2 Bass kernel for nn_GroupedConvFuseSide4.

out[b,k] = w[k,0]*side5[b,k] + w[k,1]*side4[b,k]
         + w[k,2]*side1[b,0] + w[k,3]*side2[b,0] + w[k,4]*side3[b,0] + bias[k]

Sharding: pure data parallel over batch (B=8) across 8 NeuronCores.

v6 scheme — fp16 wire, half-tile streaming, PE-heavy pipeline:
  Pixels of one image are split into CH=32 chunks of FD=8192.  Row
  r = 19*g + k gives ROWS=608 rows; 5 row-tiles (4x128 + 96) x 2
  column halves stream through SBUF as [R, 4096] 1 MB transfers with
  5-deep pools, so the free->load->use chain never starves compute
  (v5's 2 MB/3-buf tiles made loads compute-paced in the second half).
  The three singles + a ones row are RESIDENT: one [97, FD] SBUF tile
  (row 32*s+g, row 96 = ones for bias) loaded once.  Per 512-col region:
    - PE: diag(w1) @ x4  (start=True) + lsT @ [singles; ones] (stop=True).
      Both stationary operands are 128-col slices of ONE global
      [128, 1232] fp16 const blob (LS | GD | f32-scalars-bitcast), a
      single ~2.4 KB/partition DMA (v5 streamed 3 small-descriptor
      consts that clogged the HWDGE queues for ~10 us at startup).
    - DVE: one STT per [R, 2048] block: out = x5*w0 + psum (measured
      2.34 us/block; a single full pass is the DVE minimum since STT
      never gets the 2x packed mode).
  A 10-matmul warmup burst runs at kernel start so the PE HAM clock
  gate flips to 2.4 GHz before the real matmuls begin (cold PE runs
  matmuls at 634 ns vs 379 ns warm / ~218 ns issue rate).
  DMA queues balanced ~9.9 MB each: sync = blob + all x5 loads,
  scalar = all x4 loads + t4 64-part stores, gpsimd = singles +
  half-tile 1 MB stores + t4 32-part stores.  Loads are never queued
  behind stores on any queue.  All DMA partition counts are in
  {1, 32, 64, 128} (counts in 65..127 hit degenerate descriptor paths).
  Host converts to fp16 and repacks so every load is contiguous;
  output comes back fp16, upcast on host.
  Max rel err vs the f32 reference ~8e-4, well under the 2e-2 gate.
"""

import numpy as np

B, K, H, W = 8, 19, 512, 512
FD = 8192                  # pixels per chunk
CH = 32                    # chunks per image (H*W / FD)
ROWS = K * CH              # 608 packed rows per core
TILES = []                 # (row0, nrows): 4 x 128 + 1 x 96
_r = 0
while _r < ROWS:
    TILES.append((_r, min(128, ROWS - _r)))
    _r += 128
NT = len(TILES)
HW_ = FD // 2             # half-tile columns (4096)
N_CORES = 8
BLOB_W = 1232             # 608 LS + 608 GD + 10 scl-f32-raw + 6 pad

_cache = {}


def _build_program(w, b):
    import concourse.bacc as bacc
    import concourse.tile as tile
    import concourse.mybir as mybir
    from contextlib import ExitStack

    f16 = mybir.dt.float16
    f32 = mybir.dt.float32
    mult = mybir.AluOpType.mult
    add = mybir.AluOpType.add

    nc = bacc.Bacc(
        "TRN2", target_bir_lowering=False, debug=False,
        enable_asserts=False, num_devices=N_CORES,
    )

    x5_d = nc.dram_tensor("x5", [ROWS, FD], f16, kind="ExternalInput").ap()
    x4_d = nc.dram_tensor("x4", [ROWS, FD], f16, kind="ExternalInput").ap()
    s_d = nc.dram_tensor("s", [97, FD], f16, kind="ExternalInput").ap()
    out_d = nc.dram_tensor("out", [ROWS, FD], f16, kind="ExternalOutput").ap()

    # ---- one const blob; per-tile lhsT = 128-col slices of it ----
    pp = np.arange(ROWS)
    kk_g = pp % K
    gg_g = pp // K
    blob = np.zeros((128, BLOB_W), dtype=np.float16)
    # LS[32*s + g(p), p] = w[k(p), 2+s]; LS[96, p] = bias[k(p)]
    for s in range(3):
        blob[32 * s + gg_g, pp] = w[kk_g, 2 + s].astype(np.float16)
    blob[96, pp] = b[kk_g].astype(np.float16)
    # GD[r, p] = w1[k(p)] iff r == p % 128 (per-tile diagonal), at col 608+p
    blob[pp % 128, 608 + pp] = w[kk_g, 1].astype(np.float16)
    # w0 per-partition f32 scalars, raw-bitcast into fp16 cols 1216:1226
    scl = np.zeros((128, NT), dtype=np.float32)
    for t, (r0, R) in enumerate(TILES):
        rr = r0 + np.arange(R)
        scl[:R, t] = w[rr % K, 0]
    blob[:, 1216:1216 + 2 * NT] = scl.view(np.float16)
    blob_dram = nc.inline_tensor(blob, name="blobc").ap()

    with tile.TileContext(nc) as tc, ExitStack() as ctx:
        consts = ctx.enter_context(tc.tile_pool(name="consts", bufs=1))
        x5_pool = ctx.enter_context(tc.tile_pool(name="x5", bufs=5))
        x4_pool = ctx.enter_context(tc.tile_pool(name="x4", bufs=5))
        o_pool = ctx.enter_context(tc.tile_pool(name="o", bufs=3))
        o4_pool = ctx.enter_context(tc.tile_pool(name="o4", bufs=4))
        ps_pool = ctx.enter_context(tc.tile_pool(name="ps", bufs=2, space="PSUM"))

        blob_t = consts.tile([128, BLOB_W], f16, tag="blob")
        nc.sync.dma_start(out=blob_t[:], in_=blob_dram)
        s_t = consts.tile([97, FD], f16, tag="s")
        nc.gpsimd.dma_start(out=s_t[0:64], in_=s_d[0:64])
        nc.gpsimd.dma_start(out=s_t[64:96], in_=s_d[64:96])
        nc.gpsimd.dma_start(out=s_t[96:97], in_=s_d[96:97])
        scl_t = blob_t[:, 1216:1216 + 2 * NT].bitcast(f32)

        # ---- PE warmup: dense matmuls so the HAM clock gate flips to
        # 2.4 GHz before the first real matmul; results are dead writes.
        wu = consts.tile([128, 512], f16, tag="wu")
        nc.vector.memset(wu[:], 0.0)
        ps_w = ps_pool.tile([128, 2048], f32, tag="ps", name="ps_warm")
        for i in range(10):
            nc.tensor.matmul(
                ps_w[:, 0:512], wu[:, 0:128], wu[:],
                start=True, stop=True, skip_group_check=True,
            )

        # process the fiddly 96-row tail tile mid-stream (its small split
        # DMAs ride the DMA-bound middle's slack) so the kernel ends on
        # full-width tiles, and the last tile's stores fan out across the
        # by-then idle HWDGE queues.
        order = [0, 1, 4, 2, 3] if NT == 5 else list(range(NT))
        for t in order:
            r0, R = TILES[t]
            last = t == order[-1]
            diagT = blob_t[0:R, 608 + r0:608 + r0 + R]
            lsT = blob_t[0:97, r0:r0 + R]
            w0 = scl_t[0:R, t:t + 1]
            for half in range(2):
                c0 = HW_ * half
                x5h = x5_pool.tile([R, HW_], f16, tag="x5", name=f"x5_{t}_{half}")
                x4h = x4_pool.tile([R, HW_], f16, tag="x4", name=f"x4_{t}_{half}")
                if R == 128:
                    nc.sync.dma_start(out=x5h[:], in_=x5_d[r0:r0 + R, c0:c0 + HW_])
                    nc.scalar.dma_start(out=x4h[:], in_=x4_d[r0:r0 + R, c0:c0 + HW_])
                else:  # 96-row tail: keep DMA partition counts in {64,32}
                    nc.sync.dma_start(out=x5h[0:64],
                                      in_=x5_d[r0:r0 + 64, c0:c0 + HW_])
                    nc.sync.dma_start(out=x5h[64:R],
                                      in_=x5_d[r0 + 64:r0 + R, c0:c0 + HW_])
                    nc.scalar.dma_start(out=x4h[0:64],
                                        in_=x4_d[r0:r0 + 64, c0:c0 + HW_])
                    nc.scalar.dma_start(out=x4h[64:R],
                                        in_=x4_d[r0 + 64:r0 + R, c0:c0 + HW_])
                oh = (o_pool.tile([R, HW_], f16, tag="o", name=f"o_{t}_{half}")
                      if R == 128 else None)

                for sub in range(2):
                    lc = 2048 * sub           # local col in half tile
                    gc = c0 + lc              # global col
                    ps = ps_pool.tile([R, 2048], f32, tag="ps",
                                      name=f"ps_{t}_{half}_{sub}")
                    # all 4 diag matmuls back-to-back, then all 4 ls matmuls:
                    # consecutive matmuls share the stationary operand so the
                    # PE pays 2 weight swaps per block instead of 8 (a weight
                    # swap costs ~170ns of array refill per matmul).
                    for h in range(4):
                        nc.tensor.matmul(
                            ps[:, 512 * h:512 * h + 512], diagT,
                            x4h[:, lc + 512 * h:lc + 512 * h + 512],
                            start=True, stop=False, skip_group_check=True,
                        )
                    for h in range(4):
                        nc.tensor.matmul(
                            ps[:, 512 * h:512 * h + 512], lsT,
                            s_t[0:97, gc + 512 * h:gc + 512 * h + 512],
                            start=False, stop=True, skip_group_check=True,
                        )
                    if oh is not None:
                        nc.vector.scalar_tensor_tensor(
                            oh[:, lc:lc + 2048], x5h[:, lc:lc + 2048], w0,
                            ps[:], mult, add)
                    else:
                        o4 = o4_pool.tile([R, 2048], f16, tag="o4",
                                          name=f"o4_{half}_{sub}")
                        nc.vector.scalar_tensor_tensor(
                            o4[:], x5h[:, lc:lc + 2048], w0, ps[:], mult, add)
                        # tail tile: store each block immediately, split
                        # across the (by now idle) other queues
                        nc.scalar.dma_start(
                            out=out_d[r0:r0 + 64, gc:gc + 2048], in_=o4[0:64])
                        nc.gpsimd.dma_start(
                            out=out_d[r0 + 64:r0 + R, gc:gc + 2048],
                            in_=o4[64:R])
                if oh is not None:
                    if last:
                        nc.sync.dma_start(
                            out=out_d[r0:r0 + 64, c0:c0 + HW_], in_=oh[0:64])
                        nc.scalar.dma_start(
                            out=out_d[r0 + 64:r0 + R, c0:c0 + HW_],
                            in_=oh[64:R])
                    else:
                        nc.gpsimd.dma_start(out=out_d[r0:r0 + R, c0:c0 + HW_],
                                            in_=oh[:])

    nc.compile()
    return nc


def _get_program(w, b):
    key = (w.tobytes(), b.tobytes())
    if key not in _cache:
        _cache[key] = _build_program(w, b)
    return _cache[key]


def _pack_kchw(a16):
    """[K, CH, FD] fp16 -> [ROWS, FD], row = 19*g + k."""
    return np.ascontiguousarray(a16.transpose(1, 0, 2)).reshape(ROWS, FD)


def run(inputs, trace=False, tmpdir=None):
    from concourse.bass_utils import run_bass_kernel_spmd

    w = np.asarray(inputs["weight"], dtype=np.float32)
    b = np.asarray(inputs["bias"], dtype=np.float32)
    nc = _get_program(w, b)

    s1h = np.asarray(inputs["side1"]).astype(np.float16).reshape(B, CH, FD)
    s2h = np.asarray(inputs["side2"]).astype(np.float16).reshape(B, CH, FD)
    s3h = np.asarray(inputs["side3"]).astype(np.float16).reshape(B, CH, FD)
    s4h = np.asarray(inputs["side4"]).astype(np.float16).reshape(B, K, CH, FD)
    s5h = np.asarray(inputs["side5"]).astype(np.float16).reshape(B, K, CH, FD)

    in_maps = []
    for c in range(N_CORES):
        sp = np.empty((97, FD), dtype=np.float16)
        sp[0:32] = s1h[c]
        sp[32:64] = s2h[c]
        sp[64:96] = s3h[c]
        sp[96] = np.float16(1.0)
        in_maps.append({
            "x5": _pack_kchw(s5h[c]),
            "x4": _pack_kchw(s4h[c]),
            "s": sp,
        })

    res = run_bass_kernel_spmd(nc, in_maps, list(range(N_CORES)),
                               trace=trace, tmpdir=tmpdir)
    outs = []
    for c in range(N_CORES):
        o = res.results[c]["out"].reshape(CH, K, FD).transpose(1, 0, 2)
        outs.append(o.reshape(1, K, H, W).astype(np.float32))
    return np.concatenate(outs, axis=0), res


def kernel(**inputs):
    out, _ = run(inputs, trace=False)
    return out
